# revision 75
# baseline (speedup 1.0000x reference)
"""Trainium2 Bass kernel for multi-head causal self-attention.

Problem: B=4, S=2048, D=768, H=12, DH=64 (fp32).
  Q = x @ W_Q + b_Q; K, V likewise
  scores = QK^T / sqrt(DH), causal mask, softmax
  out = (probs @ V) @ W_O + b_O

Sharding over 8 cores: core c -> batch b = c//2, head-half hh = c%2
(6 heads each). Fully local compute, no collectives; the two partial
outputs per batch are summed on the host during unshard (b_O also
added on host).

Device layout is "transposed" (sequence on the free dim):
  xT    [D, S]
  QT,KT [384, S]   hk on partitions (3 chunks of 128 = 2 heads each)
  V     [S, 768]   natural layout, 128 cols per head: [1 | 0*63 | V64]
                   so P^T@[1|0|V] accumulates softmax sums on PSUM
                   partition 0 and Z on partitions 64..127 in the same
                   accumulation (partition-aligned for the normalize)
  S^T   [keys, q]  scores transposed
  Z^T   [384, S]
  outT  [D, S]     host transposes back

Performance structure (vs the naive phased version):
  * Attention runs in 512-wide q "chains" per (head-pair, q-chunk),
    processed in anti-diagonal order so projection / output-projection
    matmuls are available as PE filler against the scalar engine's exp
    latency (the exp paces the attention phase at ~1.1us/step; PE gaps
    also drop the tensor engine's HAM p-state from 2.4GHz to 1.2GHz).
  * Score matmuls are K=64 and row-paired at PE tile positions (0,0)
    and (64,0) -> the two heads' score matmuls run concurrently.
  * One exp per kt step covers both heads; diagonal steps use a
    two-region AP that skips the masked-off columns. The causal mask
    is ADDED to the score PSUM *before* the score matmuls (mask MM
    start=True clears has_written; scores accumulate on top), keeping
    it off the scores->exp critical path.
  * All PSUM->SBUF copies/bias-adds run on DVE, never ACT: the ACT
    queue is a pure exp stream (an ACT-queued copy head-of-line
    blocks the next exp behind its producer matmuls).
  * Next-chain Q/K/V projection units are prefetched as priority
    fillers (dripped 1-2/step with a flush before the normalize) so
    chain boundaries don't serialize proj -> bias-add -> scores.
  * Input DMA is the startup bottleneck (~180GB/s effective when both
    cores of an HBM stack load simultaneously): host passes all
    tensors pre-packed as SBUF images (fat descriptors), x is packed
    qc-major, and all input dma_starts issue on ONE queue in strict
    need order (ring order = issue order). dma_start issue costs
    ~0.8us, so transfers are consolidated (one per logical tensor)
    and the final qc's six output blocks ship as one DMA.
  * V blocks are zeroed (garbage bf16 stationary weights toggle the
    PE array and burn power budget -- the chip P0-downclocks to
    ~2.0GHz under sustained high power, which is also why denser
    schedules than this one measured SLOWER).
  * Output is written bf16 (error budget allows it; halves out DMA).
  * PE warmup matmuls bridge the input-DMA wait and the final
    normalize so real matmuls never run at the cold 1.2GHz p-state.
Softmax skips the max-subtraction (scores are ~N(0, 0.3)), which is
mathematically identical to the reference softmax.
"""

import numpy as np

import concourse.mybir as mybir
import concourse.tile as tile
from concourse import bacc, bass_utils

F32 = mybir.dt.float32
BF16 = mybir.dt.bfloat16

B, S, D, H, DH = 4, 2048, 768, 12, 64
HL = 6                # heads per core
HK = HL * DH          # 384
NPAIR = HL // 2       # 3 head pairs (128 partitions each)
P = 128
NDT = D // P          # 6 d-tiles
NST = S // P          # 16 key tiles
QW = 512              # q chain width (one PSUM bank)
NQC = S // QW         # 4 chains per head pair
VW = DH + 1           # 65 = V cols + ones col
VTW = 128             # V block stride per head: [1 | 0*63 | V64]
SCALE = 1.0 / 8.0     # 1/sqrt(DH)

# interleaver cost model (ns)
PEC = 1e9 / 2.4e9
ACTC = 1e9 / 1.2e9
ACT_OVH = 195.0

# anti-diagonal chain order: spreads deep (exp-heavy) chains across
# the kernel so projection/output-projection filler is available
# (a qc-major order measured ~1.5us slower: its early qc=0 phase is
# hard-PE-bound and stretches the exp stream more than the relaxed
# x-qc1 DMA deadline saves)
CHAIN_ORDER = [(0, 0), (0, 1), (1, 0), (0, 2), (1, 1), (2, 0),
               (0, 3), (1, 2), (2, 1), (1, 3), (2, 2), (2, 3)]

N_WARMUP = 26           # warmup matmuls bridging the input-DMA wait
                        # (keeps HAM warm so sQ/sK run at 2.4GHz)
HALF = S // 2         # x DMA'd in two column halves


def _np_in(a):
    import ml_dtypes
    return np.ascontiguousarray(a, dtype=np.float32).astype(ml_dtypes.bfloat16)


def build_nc():
    nc = bacc.Bacc("TRN2", target_bir_lowering=False, debug=False, num_devices=8)

    # host-packed layouts: every DRAM tensor is already the SBUF image
    # ([128 partitions, free]) so input DMAs use big contiguous
    # descriptors (the [D, x] row-major layouts give only 768B lines)
    xp = nc.dram_tensor("xp", [P, NDT * S], BF16, kind="ExternalInput").ap()
    wqp = nc.dram_tensor("wqp", [P, NDT * HK], BF16, kind="ExternalInput").ap()
    wkp = nc.dram_tensor("wkp", [P, NDT * HK], BF16, kind="ExternalInput").ap()
    wvp = nc.dram_tensor("wvp", [P, NDT * HK], BF16, kind="ExternalInput").ap()
    wop = nc.dram_tensor("wop", [P, NPAIR * D], BF16, kind="ExternalInput").ap()
    # cb16 = [causal mask block | identity], [key, q] layout
    cb16 = nc.dram_tensor("cb16", [P, 2 * P], BF16, kind="ExternalInput").ap()
    # cf32 = [b_Q | b_K] per-partition
    cf32 = nc.dram_tensor("cf32", [P, 2 * NPAIR], F32, kind="ExternalInput").ap()
    out = nc.dram_tensor("out", [D, S], BF16, kind="ExternalOutput").ap()

    EXP = mybir.ActivationFunctionType.Exp

    with tile.TileContext(nc) as tc:
        with (
            tc.tile_pool(name="big", bufs=1) as big,
            tc.tile_pool(name="wts", bufs=1) as wts,
            tc.tile_pool(name="vpool", bufs=1) as vpool,
            tc.tile_pool(name="small", bufs=1) as small,
            tc.tile_pool(name="pp", bufs=6) as pp,
            tc.tile_pool(name="ocp", bufs=4) as ocp_pool,
            tc.tile_pool(name="rrp", bufs=6) as rr_pool,
            tc.tile_pool(name="rbp", bufs=6) as rb_pool,
            tc.tile_pool(name="otp", bufs=4) as otp,
            tc.tile_pool(name="ps_s", bufs=2, space="PSUM") as ps_s,
            tc.tile_pool(name="ps_o", bufs=2, space="PSUM") as ps_o,
            tc.tile_pool(name="ps_p", bufs=2, space="PSUM") as ps_p,
        ):
            # ---- input DMAs -------------------------------------------
            # Each dma_start costs ~0.8us of fixed issue time on its
            # queue, so inputs are consolidated: one issue per logical
            # tensor. x comes in three column chunks (qc0 cols first)
            # so the first Q/K projections start after ~1/5 of the
            # input traffic. The scalar queue takes the weights (HWDGE;
            # idle before the first exp); gpsimd issues nothing (SWDGE
            # descriptor-gen gets blocked by DVE 2-port ops, and its
            # queue must stay free for the normalize broadcasts).
            # x layout is qc-major: [p, qc, dt, s] so each qc chunk is
            # one contiguous 6KB-per-partition DMA (qc0 lands first and
            # the first projections start after ~1/4 of the x traffic)
            xt_all = big.tile([P, NDT * S], BF16, tag="xt", name="xt_all")
            xqv = xt_all.rearrange("p (q t s) -> p q t s", t=NDT, s=QW)
            xpv = xp.rearrange("p (q t s) -> p q t s", t=NDT, s=QW)

            def x_ap(dt, qc):
                return xqv[:, qc, dt, :]

            def x_key_ap(dt, st):
                return xqv[:, st // 4, dt, (st % 4) * P:(st % 4 + 1) * P]
            wq_all = wts.tile([P, NDT * HK], BF16, tag="wq", name="wq_all")
            wk_all = wts.tile([P, NDT * HK], BF16, tag="wk", name="wk_all")
            wv_all = wts.tile([P, NDT * HK], BF16, tag="wv", name="wv_all")
            wq_sb = [wq_all[:, dt * HK:(dt + 1) * HK] for dt in range(NDT)]
            wk_sb = [wk_all[:, dt * HK:(dt + 1) * HK] for dt in range(NDT)]
            wv_sb = [wv_all[:, dt * HK:(dt + 1) * HK] for dt in range(NDT)]
            wo_all = wts.tile([P, NPAIR * D], BF16, tag="wo", name="wo_all")
            wo_sb = [wo_all[:, c * D:(c + 1) * D] for c in range(NPAIR)]
            consts = small.tile([P, 2 * P], BF16, tag="consts")
            mska_sb = consts[:, 0:P]
            iden_sb = consts[:, P:2 * P]
            bias_sb = small.tile([P, 2 * NPAIR], F32, tag="bias")
            bq_sb = bias_sb[:, 0:NPAIR]
            bk_sb = bias_sb[:, NPAIR:2 * NPAIR]

            # The input DMA runs at only ~180GB/s effective (both cores
            # of an HBM stack pull their inputs simultaneously), so the
            # startup is transfer-bound and ring order decides when the
            # first projections can run. Ring order follows issue
            # order; a single queue in strict need order keeps x qc1..
            # from competing with the critical set (x qc0, wq, wk).
            nc.sync.dma_start(out=xqv[:, 0], in_=xpv[:, 0])
            nc.sync.dma_start(out=wq_all, in_=wqp)
            nc.sync.dma_start(out=wk_all, in_=wkp)
            nc.sync.dma_start(out=consts, in_=cb16)
            nc.sync.dma_start(out=bias_sb, in_=cf32)
            nc.sync.dma_start(out=wv_all, in_=wvp)
            nc.sync.dma_start(out=xqv[:, 1], in_=xpv[:, 1])
            nc.sync.dma_start(out=xqv[:, 2], in_=xpv[:, 2])
            nc.sync.dma_start(out=xqv[:, 3], in_=xpv[:, 3])
            nc.sync.dma_start(out=wo_all, in_=wop)
            # touch Exp after the weight issues so the ACT table load
            # (~2.7us) overlaps the input DMA phase
            warm_sb = small.tile([1, 8], F32, tag="warm")
            nc.vector.memset(warm_sb, 1.0)
            nc.scalar.activation(warm_sb, warm_sb, EXP)

            # ---- persistent compute tiles -------------------------------
            QT = [big.tile([P, S], BF16, tag=f"qt{c}", name=f"qt{c}") for c in range(NPAIR)]
            KT = [big.tile([P, S], BF16, tag=f"kt{c}", name=f"kt{c}") for c in range(NPAIR)]
            ZT = [big.tile([P, S], BF16, tag=f"zt{c}", name=f"zt{c}") for c in range(NPAIR)]
            # warm_mm first on the DVE queue so PE warmups start ASAP
            warm_mm = small.tile([P, QW], BF16, tag="warmmm")
            nc.vector.memset(warm_mm, 0.5)
            # V blocks: ones col feeds the softmax-sums row of O. Cols
            # [1:DH) only feed O partitions 1..63 (never read) but are
            # zeroed anyway: garbage bf16 stationary weights toggle the
            # PE array cells and burn power budget (the chip
            # P0-downclocks under sustained high power). Only st 0..3
            # (needed by chain (0,0)) are zeroed before the startup
            # bias-adds; the rest follow after so they don't delay the
            # critical path on the DVE queue.
            Vt = []
            for st in range(NST):
                t = vpool.tile([P, HL * VTW], BF16, tag=f"v{st}", name=f"v{st}")
                Vt.append(t)

            def zero_v(st, eng):
                vv = Vt[st].rearrange("p (h c) -> p h c", c=VTW)
                eng.memset(Vt[st], 0.0)
                eng.memset(vv[:, :, 0:1], 1.0)

            # st 0..3 on DVE (ahead of the startup bias-adds), the rest
            # on GPSIMD which is otherwise idle until the first
            # normalize broadcast -- 24 memsets on the DVE queue would
            # head-of-line block the early V copies and proj adds
            for st in range(4):
                zero_v(st, nc.vector)
            for st in range(4, NST):
                zero_v(st, nc.gpsimd)

            # ---- startup: pipelined first Q/K projections ---------------
            # Warmup matmuls (junk, never read) bridge the input-DMA
            # wait and ramp the PE p-state; the sQ/sK matmuls start as
            # soon as the x qc0-column chunk and wq/wk land.
            # Accumulators live in ps_o (idle until the first PV).
            warm_ps = ps_p.tile([P, QW], F32, tag="psp")
            sQ = ps_o.tile([P, QW], F32, tag="o", name="sQ")
            sK = ps_o.tile([P, QW], F32, tag="o", name="sK")
            for _ in range(N_WARMUP):
                nc.tensor.matmul(
                    warm_ps, lhsT=warm_mm[:, 0:P], rhs=warm_mm,
                    start=True, stop=True,
                )
            for dt in range(NDT):
                nc.tensor.matmul(
                    sQ, lhsT=wq_sb[dt][:, 0:P], rhs=x_ap(dt, 0),
                    start=(dt == 0), stop=(dt == NDT - 1),
                )
                nc.tensor.matmul(
                    sK, lhsT=wk_sb[dt][:, 0:P], rhs=x_ap(dt, 0),
                    start=(dt == 0), stop=(dt == NDT - 1),
                )
            nc.vector.tensor_scalar_add(QT[0][:, 0:QW], sQ, bq_sb[:, 0:1])
            nc.vector.tensor_scalar_add(KT[0][:, 0:QW], sK, bk_sb[:, 0:1])

            # ---- interleaver state --------------------------------------
            state = {"pe": 0.0, "act": 0.0, "tail_done": set()}
            fillers = []      # list of (emit_fn, pe_cost)

            def emit_qk(which, pr, qc):
                w_sb, b_sb, dst = (
                    (wq_sb, bq_sb, QT) if which == "q" else (wk_sb, bk_sb, KT)
                )
                pt = ps_p.tile([P, QW], F32, tag="psp")
                for dt in range(NDT):
                    nc.tensor.matmul(
                        pt,
                        lhsT=w_sb[dt][:, pr * P:(pr + 1) * P],
                        rhs=x_ap(dt, qc),
                        start=(dt == 0),
                        stop=(dt == NDT - 1),
                    )
                # copies/bias-adds stay off ACT so its queue is a pure exp
                # stream (a proj copy on ACT head-of-line-blocks the next
                # chain's exp behind its matmuls)
                dst_ap = dst[pr][:, qc * QW:(qc + 1) * QW]
                nc.vector.tensor_scalar_add(dst_ap, pt, b_sb[:, pr:pr + 1])
                state["pe"] += 6 * QW * PEC

            def emit_v(st):
                pt = ps_p.tile([P, QW], F32, tag="psp")
                for dt in range(NDT):
                    nc.tensor.matmul(
                        pt[:, 0:HK],
                        lhsT=x_key_ap(dt, st),
                        rhs=wv_sb[dt],
                        start=(dt == 0),
                        stop=(dt == NDT - 1),
                    )
                vv = Vt[st].rearrange("p (h c) -> p h c", c=VTW)
                nc.vector.tensor_copy(
                    vv[:, :, DH:VTW],
                    pt[:, 0:HK].rearrange("p (h c) -> p h c", c=DH),
                )
                state["pe"] += 6 * HK * PEC

            # the final qc's six o-proj units are the kernel tail: their
            # copies land in one wide tile and ship as ONE dma_start
            # (six separate issues cost ~0.8us of sync-queue time each)
            ot_last = big.tile([P, NDT * QW], BF16, tag="otlast", name="ot_last")
            QC_LAST = NQC - 1
            outv = out.rearrange("(t p) s -> p t s", p=P)

            def emit_oproj(qc, dt):
                pt = ps_p.tile([P, QW], F32, tag="psp")
                for c in range(NPAIR):
                    nc.tensor.matmul(
                        pt,
                        lhsT=wo_sb[c][:, dt * P:(dt + 1) * P],
                        rhs=ZT[c][:, qc * QW:(qc + 1) * QW],
                        start=(c == 0),
                        stop=(c == NPAIR - 1),
                    )
                state["pe"] += NPAIR * QW * PEC
                if qc == QC_LAST:
                    nc.vector.tensor_copy(
                        ot_last[:, dt * QW:(dt + 1) * QW], pt)
                    state["tail_done"].add(dt)
                    # ship the first half as soon as its copies exist so
                    # the final DMA pipelines with the dt3-5 matmuls
                    if ({0, 1, 2} <= state["tail_done"]
                            and not state.get("tail_half")):
                        state["tail_half"] = True
                        nc.sync.dma_start(
                            out=outv[:, 0:3, QC_LAST * QW:(QC_LAST + 1) * QW],
                            in_=ot_last.rearrange(
                                "p (t s) -> p t s", s=QW)[:, 0:3],
                        )
                    return
                osb = otp.tile([P, QW], BF16, tag="ot")
                nc.vector.tensor_copy(osb, pt)
                # output DMA on sync only: HWDGE, and keeps the gpsimd
                # queue free for the normalize broadcasts
                nc.sync.dma_start(
                    out=out[dt * P:(dt + 1) * P, qc * QW:(qc + 1) * QW],
                    in_=osb,
                )

            def flush_oproj_tail():
                nc.sync.dma_start(
                    out=outv[:, 3:6, QC_LAST * QW:(QC_LAST + 1) * QW],
                    in_=ot_last.rearrange("p (t s) -> p t s", s=QW)[:, 3:6],
                )

            pending = []      # (ready_step, emit_fn) gated oproj fillers
            pre_q = []        # (kind, key, fn) next-chain QK/V prefetch
            state["step"] = 0
            state["force"] = False

            def pull_fillers():
                while pending and pending[0][0] <= state["step"]:
                    fillers.append(pending.pop(0)[1])
                # prefetch units first: their deadline is the next chain
                while pre_q and state["act"] > state["pe"]:
                    pre_q.pop(0)[2]()
                if state["force"] and fillers:
                    fillers.pop(0)()
                while fillers and state["act"] > state["pe"]:
                    fn = fillers.pop(0)
                    fn()

            # dependency bookkeeping: JIT-emit projections a chain needs
            qk_done = {("q", 0, 0), ("k", 0, 0)}   # done in startup block
            v_done = set()

            def enqueue_next(pr, qc):
                # prefetch: queue the proj units a future chain needs so
                # pull_fillers / the drip emits them during this chain
                for q in range(qc + 1):
                    for which in ("q", "k"):
                        if (which, pr, q) not in qk_done:
                            qk_done.add((which, pr, q))
                            pre_q.append(
                                ("qk", (pr, q),
                                 lambda which=which, pr=pr, q=q:
                                     emit_qk(which, pr, q)))
                for st in range(4 * qc + 4):
                    if st not in v_done:
                        v_done.add(st)
                        pre_q.append(
                            ("v", st, lambda st=st: emit_v(st)))

            def need_qk(pr, qc):
                # flush any not-yet-pulled prefetch units this chain needs
                i = 0
                while i < len(pre_q):
                    kind, key, fn = pre_q[i]
                    if kind == "qk" and key[0] == pr and key[1] <= qc:
                        fn()
                        pre_q.pop(i)
                    else:
                        i += 1
                for q in range(qc + 1):
                    for which in ("q", "k"):
                        if (which, pr, q) not in qk_done:
                            qk_done.add((which, pr, q))
                            emit_qk(which, pr, q)

            def need_v(kt_max):
                i = 0
                while i < len(pre_q):
                    kind, key, fn = pre_q[i]
                    if kind == "v" and key <= kt_max:
                        fn()
                        pre_q.pop(i)
                    else:
                        i += 1
                for st in range(kt_max + 1):
                    if st not in v_done:
                        v_done.add(st)
                        emit_v(st)

            def emit_pv(pr, O_ab, kt, o, nkt, pt):
                for hh in range(2):
                    h0 = (2 * pr + hh) * VTW
                    nc.tensor.matmul(
                        O_ab[hh][:, o:QW],
                        lhsT=Vt[kt][:, h0:h0 + VTW],
                        rhs=pt[:, hh * QW + o:hh * QW + QW],
                        start=(kt == 0),
                        stop=(kt == nkt - 1),
                    )

            # ---- attention chains ---------------------------------------
            def emit_chain(pr, qc, nxt=None, lazy_v=False):
                nkt = 4 * qc + 4
                need_qk(pr, qc)
                # lazy_v (first chain only): emit V st0 now and each
                # later V unit one step ahead inside the loop, instead
                # of a 5us V wall in front of the first score matmuls
                need_v(0 if lazy_v else nkt - 1)
                q0 = qc * QW
                O_ab = [ps_o.tile([P, QW], F32, tag="o", name=f"o{i}") for i in range(2)]
                for kt in range(nkt):
                    o = max(0, P * kt - q0)
                    diag = P * kt >= q0
                    w = QW - o
                    sp = ps_s.tile([P, 2 * QW], F32, tag="s")
                    # mask first: start=True clears the bank's has_written
                    # bits, the score matmuls then accumulate on top of
                    # the mask region and overwrite elsewhere. Keeps the
                    # mask matmuls off the scores->exp critical path.
                    if diag:
                        for hh in range(2):
                            nc.tensor.matmul(
                                sp[:, hh * QW + o:hh * QW + o + P],
                                lhsT=iden_sb,
                                rhs=mska_sb,
                                start=True,
                                stop=False,
                            )
                    for hh in range(2):
                        lo = hh * DH
                        nc.tensor.matmul(
                            sp[:, hh * QW + o:hh * QW + QW],
                            lhsT=KT[pr][lo:lo + DH, kt * P:(kt + 1) * P],
                            rhs=QT[pr][lo:lo + DH, q0 + o:q0 + QW],
                            start=not diag,
                            stop=True,
                        )
                    pt = pp.tile([P, 2 * QW], BF16, tag="p")
                    if o == 0:
                        # one contiguous exp for both heads
                        nc.scalar.activation(pt, sp, EXP, scale=SCALE)
                    else:
                        # two-region AP skips the o masked-off columns per
                        # head (the junk region [QW:QW+o) stays stale and
                        # is never read downstream)
                        sp3 = sp.rearrange("p (h q) -> p h q", h=2)
                        pt3 = pt.rearrange("p (h q) -> p h q", h=2)
                        nc.scalar.activation(
                            pt3[:, :, o:QW], sp3[:, :, o:QW], EXP, scale=SCALE
                        )
                    if lazy_v and kt + 1 < nkt:
                        need_v(kt + 1)
                    emit_pv(pr, O_ab, kt, o, nkt, pt)
                    state["pe"] += (3 * w + (2 * P if diag else 0)) * PEC
                    state["act"] += 2 * (QW - o) * ACTC + ACT_OVH
                    state["step"] += 1
                    # forced drip: units the NEXT chain needs must land
                    # before it starts, surplus or not (short chains have
                    # no ACT surplus to absorb them at the boundary);
                    # drip 2/step when there are more units than steps
                    if kt < nkt - 1:
                        n_drip = 2 if len(pre_q) > nkt - 1 - kt else 1
                        for _ in range(min(n_drip, len(pre_q))):
                            pre_q.pop(0)[2]()
                        pull_fillers()
                # flush the next chain's remaining QK units BEFORE the
                # normalize so their DVE bias-adds queue ahead of the
                # recip/mul chain (else the next chain's scores wait for
                # adds stuck behind the normalize = multi-us ACT gap)
                if nxt is not None:
                    need_qk(*nxt)
                # normalize: ZT = O[0:64] * 1/sums, sums = O[64].
                # Emission order pipelines the two heads across DVE/GPSIMD
                # (DVE: recip0, recip1, mul0, mul1; GPS: bc0, bc1) so the
                # O banks free ~1.3us earlier than recip/bc/mul per head.
                rrs = []
                for hh in range(2):
                    rrt = rr_pool.tile([1, QW], F32, tag="rr")
                    nc.vector.reciprocal_approx_fast(out=rrt, in_=O_ab[hh][0:1, :])
                    rrs.append(rrt)
                rbs = []
                for hh in range(2):
                    rb = rb_pool.tile([P, QW], F32, tag="rb")
                    nc.gpsimd.partition_broadcast(rb, rrs[hh])
                    rbs.append(rb)
                for hh in range(2):
                    lo = hh * DH
                    nc.vector.tensor_mul(
                        ZT[pr][lo:lo + DH, q0:q0 + QW], O_ab[hh][DH:P, :],
                        rbs[hh][DH:P, :]
                    )
                pull_fillers()

            # o-proj units for qc become pullable a few attention steps
            # after chain (2, qc)'s normalize was emitted
            for ci, (pr, qc) in enumerate(CHAIN_ORDER):
                nxt = CHAIN_ORDER[ci + 1] if ci + 1 < len(CHAIN_ORDER) else None
                if nxt is not None:
                    # prefetch the NEXT chain's projections as priority
                    # fillers pulled/dripped during this chain
                    enqueue_next(*nxt)
                if ci >= len(CHAIN_ORDER) - 2:
                    # force oproj drainage through the last chains so the
                    # tail only holds the final qc's units
                    state["force"] = True
                emit_chain(pr, qc, nxt, lazy_v=(ci == 0))
                if pr == 2:
                    for dt in range(NDT):
                        pending.append(
                            (state["step"] + 4,
                             lambda qc=qc, dt=dt: emit_oproj(qc, dt))
                        )

            # junk warmups keep HAM at full clock through the final
            # normalize (DVE/GPSIMD) so the tail o-proj matmuls don't
            # run at the throttled half clock
            for _ in range(10):
                nc.tensor.matmul(
                    warm_ps, lhsT=warm_mm[:, 0:P], rhs=warm_mm,
                    start=True, stop=True,
                )
            # drain remaining fillers (final o-proj blocks)
            for _, _, fn in pre_q:
                fn()
            for _, fn in pending:
                fillers.append(fn)
            for fn in fillers:
                fn()
            flush_oproj_tail()

    nc.compile()
    return nc


_NC_CACHE = {}


def _get_nc():
    if "nc" not in _NC_CACHE:
        _NC_CACHE["nc"] = build_nc()
    return _NC_CACHE["nc"]


def _pack(a):
    """[NT*P, F] row-major -> SBUF image [P, NT*F]."""
    nt = a.shape[0] // P
    return np.ascontiguousarray(
        a.reshape(nt, P, a.shape[1]).transpose(1, 0, 2).reshape(P, -1))


def make_in_maps(x, W_Q, W_K, W_V, W_O, b_Q, b_K, b_V, b_O):
    mask_add = np.tril(np.full((P, P), -1e4, np.float32), k=-1)
    identity = np.eye(P, dtype=np.float32)
    cb16 = _np_in(np.concatenate([mask_add, identity], axis=1))
    in_maps = []
    for c in range(8):
        b, hh = divmod(c, 2)
        hs = slice(HL * hh, HL * hh + HL)
        bqp = np.asarray(b_Q[hs], np.float32).reshape(HK).reshape(NPAIR, P).T
        bkp = np.asarray(b_K[hs], np.float32).reshape(HK).reshape(NPAIR, P).T
        in_maps.append({
            # qc-major x packing: [p, qc, dt, s] (6KB contiguous per
            # (p, qc) so each qc chunk DMAs with fat descriptors)
            "xp": np.ascontiguousarray(
                _np_in(np.asarray(x[b]).T)
                .reshape(NDT, P, NQC, QW).transpose(1, 2, 0, 3)
                .reshape(P, NDT * S)),
            "wqp": _pack(_np_in(
                np.asarray(W_Q[hs]).transpose(1, 0, 2).reshape(D, HK))),
            "wkp": _pack(_np_in(
                np.asarray(W_K[hs]).transpose(1, 0, 2).reshape(D, HK))),
            "wvp": _pack(_np_in(
                np.asarray(W_V[hs]).transpose(1, 0, 2).reshape(D, HK))),
            "wop": _pack(_np_in(np.asarray(W_O[hs]).reshape(HK, D))),
            "cb16": cb16,
            "cf32": np.ascontiguousarray(
                np.concatenate([bqp, bkp], axis=1), np.float32),
        })
    return in_maps


def run(inputs, trace=False):
    nc = _get_nc()
    in_maps = make_in_maps(**inputs)
    res = bass_utils.run_bass_kernel_spmd(
        nc, in_maps, core_ids=list(range(8)), trace=trace,
        **({"trace_cores": [0]} if trace else {}),
    )
    outs = [np.asarray(r["out"], dtype=np.float32) for r in res.results]
    const = (np.asarray(inputs["b_O"], np.float32)
             + np.einsum("hk,hkd->d", np.asarray(inputs["b_V"], np.float32),
                         np.asarray(inputs["W_O"], np.float32)))
    full = np.empty((B, S, D), np.float32)
    for b in range(B):
        full[b] = (outs[2 * b] + outs[2 * b + 1]).T + const
    return full, res


def kernel(**inputs):
    full, _ = run(inputs)
    return full



# revision 79
# speedup vs baseline: 1.1908x; 1.1908x over previous
"""Trainium2 Bass kernel for multi-head causal self-attention.

Problem: B=4, S=2048, D=768, H=12, DH=64 (fp32).
  Q = x @ W_Q + b_Q; K, V likewise
  scores = QK^T / sqrt(DH), causal mask, softmax
  out = (probs @ V) @ W_O + b_O

Sharding over 8 cores: core c -> batch b = c//2, head-half hh = c%2
(6 heads each). Fully local compute, no collectives; the two partial
outputs per batch are summed on the host during unshard (b_O also
added on host).

Device layout is "transposed" (sequence on the free dim):
  xT    [D, S]
  QT,KT [384, S]   hk on partitions (3 chunks of 128 = 2 heads each)
  V     [S, 768]   natural layout, 128 cols per head: [1 | 0*63 | V64]
                   so P^T@[1|0|V] accumulates softmax sums on PSUM
                   partition 0 and Z on partitions 64..127 in the same
                   accumulation (partition-aligned for the normalize)
  S^T   [keys, q]  scores transposed
  Z^T   [384, S]
  outT  [D, S]     host transposes back

Performance structure (vs the naive phased version):
  * Attention runs in 512-wide q "chains" per (head-pair, q-chunk),
    processed in anti-diagonal order so projection / output-projection
    matmuls are available as PE filler against the scalar engine's exp
    latency (the exp paces the attention phase at ~1.1us/step; PE gaps
    also drop the tensor engine's HAM p-state from 2.4GHz to 1.2GHz).
  * Score matmuls are K=64 and row-paired at PE tile positions (0,0)
    and (64,0) -> the two heads' score matmuls run concurrently.
  * One exp per kt step covers both heads; diagonal steps use a
    two-region AP that skips the masked-off columns. The causal mask
    is ADDED to the score PSUM *before* the score matmuls (mask MM
    start=True clears has_written; scores accumulate on top), keeping
    it off the scores->exp critical path.
  * All PSUM->SBUF copies/bias-adds run on DVE, never ACT: the ACT
    queue is a pure exp stream (an ACT-queued copy head-of-line
    blocks the next exp behind its producer matmuls).
  * Next-chain Q/K/V projection units are prefetched as priority
    fillers (dripped 1-2/step with a flush before the normalize) so
    chain boundaries don't serialize proj -> bias-add -> scores.
  * Input DMA is the startup bottleneck (~180GB/s effective when both
    cores of an HBM stack load simultaneously): host passes all
    tensors pre-packed as SBUF images (fat descriptors), x is packed
    qc-major, and all input dma_starts issue on ONE queue in strict
    need order (ring order = issue order). dma_start issue costs
    ~0.8us, so transfers are consolidated (one per logical tensor)
    and the final qc's six output blocks ship as one DMA.
  * V blocks are zeroed (garbage bf16 stationary weights toggle the
    PE array and burn power budget -- the chip P0-downclocks to
    ~2.0GHz under sustained high power, which is also why denser
    schedules than this one measured SLOWER).
  * Output is written bf16 (error budget allows it; halves out DMA).
  * PE warmup matmuls bridge the input-DMA wait and the final
    normalize so real matmuls never run at the cold 1.2GHz p-state.
Softmax skips the max-subtraction (scores are ~N(0, 0.3)), which is
mathematically identical to the reference softmax.
"""

import numpy as np

import concourse.mybir as mybir
import concourse.tile as tile
from concourse import bacc, bass_utils

F32 = mybir.dt.float32
BF16 = mybir.dt.bfloat16

B, S, D, H, DH = 4, 2048, 768, 12, 64
HL = 6                # heads per core
HK = HL * DH          # 384
NPAIR = HL // 2       # 3 head pairs (128 partitions each)
P = 128
NDT = D // P          # 6 d-tiles
NST = S // P          # 16 key tiles
QW = 512              # q chain width (one PSUM bank)
NQC = S // QW         # 4 chains per head pair
VW = DH + 1           # 65 = V cols + ones col
VTW = 128             # V block stride per head: [1 | 0*63 | V64]
SCALE = 1.0 / 8.0     # 1/sqrt(DH)

# interleaver cost model (ns)
PEC = 1e9 / 2.4e9
ACTC = 1e9 / 1.2e9
ACT_OVH = 195.0

# anti-diagonal chain order: spreads deep (exp-heavy) chains across
# the kernel so projection/output-projection filler is available
# (a qc-major order measured ~1.5us slower: its early qc=0 phase is
# hard-PE-bound and stretches the exp stream more than the relaxed
# x-qc1 DMA deadline saves)
CHAIN_ORDER = [(0, 0), (0, 1), (1, 0), (0, 2), (1, 1), (2, 0),
               (0, 3), (1, 2), (2, 1), (1, 3), (2, 2), (2, 3)]

N_WARMUP = 26           # warmup matmuls bridging the input-DMA wait
                        # (keeps HAM warm so sQ/sK run at 2.4GHz)
HALF = S // 2         # x DMA'd in two column halves


def _np_in(a):
    import ml_dtypes
    return np.ascontiguousarray(a, dtype=np.float32).astype(ml_dtypes.bfloat16)


def build_nc():
    nc = bacc.Bacc("TRN2", target_bir_lowering=False, debug=False, num_devices=8)

    # host-packed layouts: every DRAM tensor is already the SBUF image
    # ([128 partitions, free]) so input DMAs use big contiguous
    # descriptors (the [D, x] row-major layouts give only 768B lines)
    xp = nc.dram_tensor("xp", [P, NDT * S], BF16, kind="ExternalInput").ap()
    wqp = nc.dram_tensor("wqp", [P, NDT * HK], BF16, kind="ExternalInput").ap()
    wkp = nc.dram_tensor("wkp", [P, NDT * HK], BF16, kind="ExternalInput").ap()
    wvp = nc.dram_tensor("wvp", [P, NDT * HK], BF16, kind="ExternalInput").ap()
    wop = nc.dram_tensor("wop", [P, NPAIR * D], BF16, kind="ExternalInput").ap()
    # cb16 = [causal mask block | identity], [key, q] layout
    cb16 = nc.dram_tensor("cb16", [P, 2 * P], BF16, kind="ExternalInput").ap()
    # cf32 = [b_Q | b_K] per-partition
    cf32 = nc.dram_tensor("cf32", [P, 2 * NPAIR], F32, kind="ExternalInput").ap()
    out = nc.dram_tensor("out", [D, S], BF16, kind="ExternalOutput").ap()

    EXP = mybir.ActivationFunctionType.Exp

    with tile.TileContext(nc) as tc:
        with (
            tc.tile_pool(name="big", bufs=1) as big,
            tc.tile_pool(name="wts", bufs=1) as wts,
            tc.tile_pool(name="vpool", bufs=1) as vpool,
            tc.tile_pool(name="small", bufs=1) as small,
            tc.tile_pool(name="pp", bufs=6) as pp,
            tc.tile_pool(name="ocp", bufs=4) as ocp_pool,
            tc.tile_pool(name="rrp", bufs=6) as rr_pool,
            tc.tile_pool(name="rbp", bufs=6) as rb_pool,
            tc.tile_pool(name="otp", bufs=4) as otp,
            tc.tile_pool(name="ps_s", bufs=2, space="PSUM") as ps_s,
            tc.tile_pool(name="ps_o", bufs=2, space="PSUM") as ps_o,
            tc.tile_pool(name="ps_p", bufs=2, space="PSUM") as ps_p,
        ):
            # ---- input DMAs -------------------------------------------
            # Each dma_start costs ~0.8us of fixed issue time on its
            # queue, so inputs are consolidated: one issue per logical
            # tensor. x comes in three column chunks (qc0 cols first)
            # so the first Q/K projections start after ~1/5 of the
            # input traffic. The scalar queue takes the weights (HWDGE;
            # idle before the first exp); gpsimd issues nothing (SWDGE
            # descriptor-gen gets blocked by DVE 2-port ops, and its
            # queue must stay free for the normalize broadcasts).
            # x layout is qc-major: [p, qc, dt, s] so each qc chunk is
            # one contiguous 6KB-per-partition DMA (qc0 lands first and
            # the first projections start after ~1/4 of the x traffic)
            xt_all = big.tile([P, NDT * S], BF16, tag="xt", name="xt_all")
            xqv = xt_all.rearrange("p (q t s) -> p q t s", t=NDT, s=QW)
            xpv = xp.rearrange("p (q t s) -> p q t s", t=NDT, s=QW)

            def x_ap(dt, qc):
                return xqv[:, qc, dt, :]

            def x_key_ap(dt, st):
                return xqv[:, st // 4, dt, (st % 4) * P:(st % 4 + 1) * P]
            wq_all = wts.tile([P, NDT * HK], BF16, tag="wq", name="wq_all")
            wk_all = wts.tile([P, NDT * HK], BF16, tag="wk", name="wk_all")
            wv_all = wts.tile([P, NDT * HK], BF16, tag="wv", name="wv_all")
            wq_sb = [wq_all[:, dt * HK:(dt + 1) * HK] for dt in range(NDT)]
            wk_sb = [wk_all[:, dt * HK:(dt + 1) * HK] for dt in range(NDT)]
            wv_sb = [wv_all[:, dt * HK:(dt + 1) * HK] for dt in range(NDT)]
            wo_all = wts.tile([P, NPAIR * D], BF16, tag="wo", name="wo_all")
            wo_sb = [wo_all[:, c * D:(c + 1) * D] for c in range(NPAIR)]
            consts = small.tile([P, 2 * P], BF16, tag="consts")
            mska_sb = consts[:, 0:P]
            iden_sb = consts[:, P:2 * P]
            bias_sb = small.tile([P, 2 * NPAIR], F32, tag="bias")
            bq_sb = bias_sb[:, 0:NPAIR]
            bk_sb = bias_sb[:, NPAIR:2 * NPAIR]

            # The input DMA runs at only ~180GB/s effective (both cores
            # of an HBM stack pull their inputs simultaneously), so the
            # startup is transfer-bound and ring order decides when the
            # first projections can run. Ring order follows issue
            # order; a single queue in strict need order keeps x qc1..
            # from competing with the critical set (x qc0, wq, wk).
            nc.sync.dma_start(out=xqv[:, 0], in_=xpv[:, 0])
            nc.sync.dma_start(out=wq_all, in_=wqp)
            nc.sync.dma_start(out=wk_all, in_=wkp)
            nc.sync.dma_start(out=consts, in_=cb16)
            nc.sync.dma_start(out=bias_sb, in_=cf32)
            nc.sync.dma_start(out=wv_all, in_=wvp)
            nc.sync.dma_start(out=xqv[:, 1], in_=xpv[:, 1])
            nc.sync.dma_start(out=xqv[:, 2], in_=xpv[:, 2])
            nc.sync.dma_start(out=xqv[:, 3], in_=xpv[:, 3])
            nc.sync.dma_start(out=wo_all, in_=wop)
            # touch Exp after the weight issues so the ACT table load
            # (~2.7us) overlaps the input DMA phase
            warm_sb = small.tile([1, 8], F32, tag="warm")
            nc.vector.memset(warm_sb, 1.0)
            nc.scalar.activation(warm_sb, warm_sb, EXP)

            # ---- persistent compute tiles -------------------------------
            QT = [big.tile([P, S], BF16, tag=f"qt{c}", name=f"qt{c}") for c in range(NPAIR)]
            KT = [big.tile([P, S], BF16, tag=f"kt{c}", name=f"kt{c}") for c in range(NPAIR)]
            ZT = [big.tile([P, S], BF16, tag=f"zt{c}", name=f"zt{c}") for c in range(NPAIR)]
            # warm_mm first on the DVE queue so PE warmups start ASAP
            warm_mm = small.tile([P, QW], BF16, tag="warmmm")
            nc.vector.memset(warm_mm, 0.5)
            # V blocks: ones col feeds the softmax-sums row of O. Cols
            # [1:DH) only feed O partitions 1..63 (never read) but are
            # zeroed anyway: garbage bf16 stationary weights toggle the
            # PE array cells and burn power budget (the chip
            # P0-downclocks under sustained high power). Only st 0..3
            # (needed by chain (0,0)) are zeroed before the startup
            # bias-adds; the rest follow after so they don't delay the
            # critical path on the DVE queue.
            Vt = []
            for st in range(NST):
                t = vpool.tile([P, HL * VTW], BF16, tag=f"v{st}", name=f"v{st}")
                Vt.append(t)

            def zero_v(st, eng):
                vv = Vt[st].rearrange("p (h c) -> p h c", c=VTW)
                eng.memset(Vt[st], 0.0)
                eng.memset(vv[:, :, 0:1], 1.0)

            # st 0..3 on DVE (ahead of the startup bias-adds), the rest
            # on GPSIMD which is otherwise idle until the first
            # normalize broadcast -- 24 memsets on the DVE queue would
            # head-of-line block the early V copies and proj adds
            for st in range(4):
                zero_v(st, nc.vector)
            for st in range(4, NST):
                zero_v(st, nc.gpsimd)

            # ---- startup: pipelined first Q/K projections ---------------
            # Warmup matmuls (junk, never read) bridge the input-DMA
            # wait and ramp the PE p-state; the sQ/sK matmuls start as
            # soon as the x qc0-column chunk and wq/wk land.
            # Accumulators live in ps_o (idle until the first PV).
            warm_ps = ps_p.tile([P, QW], F32, tag="psp")
            sQ = ps_o.tile([P, QW], F32, tag="o", name="sQ")
            sK = ps_o.tile([P, QW], F32, tag="o", name="sK")
            for _ in range(N_WARMUP):
                nc.tensor.matmul(
                    warm_ps, lhsT=warm_mm[:, 0:P], rhs=warm_mm,
                    start=True, stop=True,
                )
            for dt in range(NDT):
                nc.tensor.matmul(
                    sQ, lhsT=wq_sb[dt][:, 0:P], rhs=x_ap(dt, 0),
                    start=(dt == 0), stop=(dt == NDT - 1),
                )
                nc.tensor.matmul(
                    sK, lhsT=wk_sb[dt][:, 0:P], rhs=x_ap(dt, 0),
                    start=(dt == 0), stop=(dt == NDT - 1),
                )
            nc.vector.tensor_scalar_add(QT[0][:, 0:QW], sQ, bq_sb[:, 0:1])
            nc.vector.tensor_scalar_add(KT[0][:, 0:QW], sK, bk_sb[:, 0:1])

            # ---- interleaver state --------------------------------------
            state = {"pe": 0.0, "act": 0.0, "tail_done": set()}
            fillers = []      # list of (emit_fn, pe_cost)

            def emit_qk(which, pr, qc):
                w_sb, b_sb, dst = (
                    (wq_sb, bq_sb, QT) if which == "q" else (wk_sb, bk_sb, KT)
                )
                pt = ps_p.tile([P, QW], F32, tag="psp")
                for dt in range(NDT):
                    nc.tensor.matmul(
                        pt,
                        lhsT=w_sb[dt][:, pr * P:(pr + 1) * P],
                        rhs=x_ap(dt, qc),
                        start=(dt == 0),
                        stop=(dt == NDT - 1),
                    )
                # copies/bias-adds stay off ACT so its queue is a pure exp
                # stream (a proj copy on ACT head-of-line-blocks the next
                # chain's exp behind its matmuls)
                dst_ap = dst[pr][:, qc * QW:(qc + 1) * QW]
                nc.vector.tensor_scalar_add(dst_ap, pt, b_sb[:, pr:pr + 1])
                state["pe"] += 6 * QW * PEC

            def emit_v(st):
                pt = ps_p.tile([P, QW], F32, tag="psp")
                for dt in range(NDT):
                    nc.tensor.matmul(
                        pt[:, 0:HK],
                        lhsT=x_key_ap(dt, st),
                        rhs=wv_sb[dt],
                        start=(dt == 0),
                        stop=(dt == NDT - 1),
                    )
                vv = Vt[st].rearrange("p (h c) -> p h c", c=VTW)
                nc.vector.tensor_copy(
                    vv[:, :, DH:VTW],
                    pt[:, 0:HK].rearrange("p (h c) -> p h c", c=DH),
                )
                state["pe"] += 6 * HK * PEC

            # the final qc's six o-proj units are the kernel tail: their
            # copies land in one wide tile and ship as ONE dma_start
            # (six separate issues cost ~0.8us of sync-queue time each)
            ot_last = big.tile([P, NDT * QW], BF16, tag="otlast", name="ot_last")
            QC_LAST = NQC - 1
            outv = out.rearrange("(t p) s -> p t s", p=P)

            def emit_oproj(qc, dt):
                pt = ps_p.tile([P, QW], F32, tag="psp")
                for c in range(NPAIR):
                    nc.tensor.matmul(
                        pt,
                        lhsT=wo_sb[c][:, dt * P:(dt + 1) * P],
                        rhs=ZT[c][:, qc * QW:(qc + 1) * QW],
                        start=(c == 0),
                        stop=(c == NPAIR - 1),
                    )
                state["pe"] += NPAIR * QW * PEC
                if qc == QC_LAST:
                    nc.vector.tensor_copy(
                        ot_last[:, dt * QW:(dt + 1) * QW], pt)
                    return
                osb = otp.tile([P, QW], BF16, tag="ot")
                nc.vector.tensor_copy(osb, pt)
                # output DMA on sync only: HWDGE, and keeps the gpsimd
                # queue free for the normalize broadcasts
                nc.sync.dma_start(
                    out=out[dt * P:(dt + 1) * P, qc * QW:(qc + 1) * QW],
                    in_=osb,
                )

            def flush_oproj_tail():
                nc.sync.dma_start(
                    out=outv[:, :, QC_LAST * QW:(QC_LAST + 1) * QW],
                    in_=ot_last.rearrange("p (t s) -> p t s", s=QW),
                )

            pending = []      # (ready_step, emit_fn) gated oproj fillers
            pre_q = []        # (kind, key, fn) next-chain QK/V prefetch
            state["step"] = 0
            state["force"] = False

            def pull_fillers():
                while pending and pending[0][0] <= state["step"]:
                    fillers.append(pending.pop(0)[1])
                # prefetch units first: their deadline is the next chain
                while pre_q and state["act"] > state["pe"]:
                    pre_q.pop(0)[2]()
                if state["force"] and fillers:
                    fillers.pop(0)()
                while fillers and state["act"] > state["pe"]:
                    fn = fillers.pop(0)
                    fn()

            # dependency bookkeeping: JIT-emit projections a chain needs
            qk_done = {("q", 0, 0), ("k", 0, 0)}   # done in startup block
            v_done = set()

            def enqueue_next(pr, qc):
                # prefetch: queue the proj units a future chain needs so
                # pull_fillers / the drip emits them during this chain
                for q in range(qc + 1):
                    for which in ("q", "k"):
                        if (which, pr, q) not in qk_done:
                            qk_done.add((which, pr, q))
                            pre_q.append(
                                ("qk", (pr, q),
                                 lambda which=which, pr=pr, q=q:
                                     emit_qk(which, pr, q)))
                for st in range(4 * qc + 4):
                    if st not in v_done:
                        v_done.add(st)
                        pre_q.append(
                            ("v", st, lambda st=st: emit_v(st)))

            def need_qk(pr, qc):
                # flush any not-yet-pulled prefetch units this chain needs
                i = 0
                while i < len(pre_q):
                    kind, key, fn = pre_q[i]
                    if kind == "qk" and key[0] == pr and key[1] <= qc:
                        fn()
                        pre_q.pop(i)
                    else:
                        i += 1
                for q in range(qc + 1):
                    for which in ("q", "k"):
                        if (which, pr, q) not in qk_done:
                            qk_done.add((which, pr, q))
                            emit_qk(which, pr, q)

            def need_v(kt_max):
                i = 0
                while i < len(pre_q):
                    kind, key, fn = pre_q[i]
                    if kind == "v" and key <= kt_max:
                        fn()
                        pre_q.pop(i)
                    else:
                        i += 1
                for st in range(kt_max + 1):
                    if st not in v_done:
                        v_done.add(st)
                        emit_v(st)

            def emit_pv(pr, O_ab, kt, o, nkt, pt):
                for hh in range(2):
                    h0 = (2 * pr + hh) * VTW
                    nc.tensor.matmul(
                        O_ab[hh][:, o:QW],
                        lhsT=Vt[kt][:, h0:h0 + VTW],
                        rhs=pt[:, hh * QW + o:hh * QW + QW],
                        start=(kt == 0),
                        stop=(kt == nkt - 1),
                    )

            # ---- attention chains ---------------------------------------
            def emit_chain(pr, qc, nxt=None, lazy_v=False):
                nkt = 4 * qc + 4
                need_qk(pr, qc)
                need_v(nkt - 1)
                q0 = qc * QW
                O_ab = [ps_o.tile([P, QW], F32, tag="o", name=f"o{i}") for i in range(2)]
                for kt in range(nkt):
                    o = max(0, P * kt - q0)
                    diag = P * kt >= q0
                    w = QW - o
                    sp = ps_s.tile([P, 2 * QW], F32, tag="s")
                    # mask first: start=True clears the bank's has_written
                    # bits, the score matmuls then accumulate on top of
                    # the mask region and overwrite elsewhere. Keeps the
                    # mask matmuls off the scores->exp critical path.
                    if diag:
                        for hh in range(2):
                            nc.tensor.matmul(
                                sp[:, hh * QW + o:hh * QW + o + P],
                                lhsT=iden_sb,
                                rhs=mska_sb,
                                start=True,
                                stop=False,
                            )
                    for hh in range(2):
                        lo = hh * DH
                        nc.tensor.matmul(
                            sp[:, hh * QW + o:hh * QW + QW],
                            lhsT=KT[pr][lo:lo + DH, kt * P:(kt + 1) * P],
                            rhs=QT[pr][lo:lo + DH, q0 + o:q0 + QW],
                            start=not diag,
                            stop=True,
                        )
                    pt = pp.tile([P, 2 * QW], BF16, tag="p")
                    if o == 0:
                        # one contiguous exp for both heads
                        nc.scalar.activation(pt, sp, EXP, scale=SCALE)
                    else:
                        # two-region AP skips the o masked-off columns per
                        # head (the junk region [QW:QW+o) stays stale and
                        # is never read downstream)
                        sp3 = sp.rearrange("p (h q) -> p h q", h=2)
                        pt3 = pt.rearrange("p (h q) -> p h q", h=2)
                        nc.scalar.activation(
                            pt3[:, :, o:QW], sp3[:, :, o:QW], EXP, scale=SCALE
                        )
                    emit_pv(pr, O_ab, kt, o, nkt, pt)
                    state["pe"] += (3 * w + (2 * P if diag else 0)) * PEC
                    state["act"] += 2 * (QW - o) * ACTC + ACT_OVH
                    state["step"] += 1
                    # forced drip: units the NEXT chain needs must land
                    # before it starts, surplus or not (short chains have
                    # no ACT surplus to absorb them at the boundary);
                    # drip 2/step when there are more units than steps
                    if kt < nkt - 1:
                        n_drip = 2 if len(pre_q) > nkt - 1 - kt else 1
                        for _ in range(min(n_drip, len(pre_q))):
                            pre_q.pop(0)[2]()
                        pull_fillers()
                # flush the next chain's remaining QK units BEFORE the
                # normalize so their DVE bias-adds queue ahead of the
                # recip/mul chain (else the next chain's scores wait for
                # adds stuck behind the normalize = multi-us ACT gap)
                if nxt is not None:
                    need_qk(*nxt)
                # normalize: ZT = O[0:64] * 1/sums, sums = O[64].
                # Emission order pipelines the two heads across DVE/GPSIMD
                # (DVE: recip0, recip1, mul0, mul1; GPS: bc0, bc1) so the
                # O banks free ~1.3us earlier than recip/bc/mul per head.
                rrs = []
                for hh in range(2):
                    rrt = rr_pool.tile([1, QW], F32, tag="rr")
                    nc.vector.reciprocal_approx_fast(out=rrt, in_=O_ab[hh][0:1, :])
                    rrs.append(rrt)
                rbs = []
                for hh in range(2):
                    rb = rb_pool.tile([P, QW], F32, tag="rb")
                    nc.gpsimd.partition_broadcast(rb, rrs[hh])
                    rbs.append(rb)
                for hh in range(2):
                    lo = hh * DH
                    nc.vector.tensor_mul(
                        ZT[pr][lo:lo + DH, q0:q0 + QW], O_ab[hh][DH:P, :],
                        rbs[hh][DH:P, :]
                    )
                pull_fillers()

            # o-proj units for qc become pullable a few attention steps
            # after chain (2, qc)'s normalize was emitted
            for ci, (pr, qc) in enumerate(CHAIN_ORDER):
                nxt = CHAIN_ORDER[ci + 1] if ci + 1 < len(CHAIN_ORDER) else None
                if nxt is not None:
                    # prefetch the NEXT chain's projections as priority
                    # fillers pulled/dripped during this chain
                    enqueue_next(*nxt)
                if ci >= len(CHAIN_ORDER) - 2:
                    # force oproj drainage through the last chains so the
                    # tail only holds the final qc's units
                    state["force"] = True
                emit_chain(pr, qc, nxt, lazy_v=(ci == 0))
                if pr == 2:
                    for dt in range(NDT):
                        pending.append(
                            (state["step"] + 4,
                             lambda qc=qc, dt=dt: emit_oproj(qc, dt))
                        )

            # junk warmups keep HAM at full clock through the final
            # normalize (DVE/GPSIMD) so the tail o-proj matmuls don't
            # run at the throttled half clock
            for _ in range(10):
                nc.tensor.matmul(
                    warm_ps, lhsT=warm_mm[:, 0:P], rhs=warm_mm,
                    start=True, stop=True,
                )
            # drain remaining fillers (final o-proj blocks)
            for _, _, fn in pre_q:
                fn()
            for _, fn in pending:
                fillers.append(fn)
            for fn in fillers:
                fn()
            flush_oproj_tail()

    nc.compile()
    return nc


_NC_CACHE = {}


def _get_nc():
    if "nc" not in _NC_CACHE:
        _NC_CACHE["nc"] = build_nc()
    return _NC_CACHE["nc"]


def _pack(a):
    """[NT*P, F] row-major -> SBUF image [P, NT*F]."""
    nt = a.shape[0] // P
    return np.ascontiguousarray(
        a.reshape(nt, P, a.shape[1]).transpose(1, 0, 2).reshape(P, -1))


def make_in_maps(x, W_Q, W_K, W_V, W_O, b_Q, b_K, b_V, b_O):
    mask_add = np.tril(np.full((P, P), -1e4, np.float32), k=-1)
    identity = np.eye(P, dtype=np.float32)
    cb16 = _np_in(np.concatenate([mask_add, identity], axis=1))
    in_maps = []
    for c in range(8):
        b, hh = divmod(c, 2)
        hs = slice(HL * hh, HL * hh + HL)
        bqp = np.asarray(b_Q[hs], np.float32).reshape(HK).reshape(NPAIR, P).T
        bkp = np.asarray(b_K[hs], np.float32).reshape(HK).reshape(NPAIR, P).T
        in_maps.append({
            # qc-major x packing: [p, qc, dt, s] (6KB contiguous per
            # (p, qc) so each qc chunk DMAs with fat descriptors)
            "xp": np.ascontiguousarray(
                _np_in(np.asarray(x[b]).T)
                .reshape(NDT, P, NQC, QW).transpose(1, 2, 0, 3)
                .reshape(P, NDT * S)),
            "wqp": _pack(_np_in(
                np.asarray(W_Q[hs]).transpose(1, 0, 2).reshape(D, HK))),
            "wkp": _pack(_np_in(
                np.asarray(W_K[hs]).transpose(1, 0, 2).reshape(D, HK))),
            "wvp": _pack(_np_in(
                np.asarray(W_V[hs]).transpose(1, 0, 2).reshape(D, HK))),
            "wop": _pack(_np_in(np.asarray(W_O[hs]).reshape(HK, D))),
            "cb16": cb16,
            "cf32": np.ascontiguousarray(
                np.concatenate([bqp, bkp], axis=1), np.float32),
        })
    return in_maps


def run(inputs, trace=False):
    nc = _get_nc()
    in_maps = make_in_maps(**inputs)
    res = bass_utils.run_bass_kernel_spmd(
        nc, in_maps, core_ids=list(range(8)), trace=trace,
        **({"trace_cores": [0]} if trace else {}),
    )
    outs = [np.asarray(r["out"], dtype=np.float32) for r in res.results]
    const = (np.asarray(inputs["b_O"], np.float32)
             + np.einsum("hk,hkd->d", np.asarray(inputs["b_V"], np.float32),
                         np.asarray(inputs["W_O"], np.float32)))
    full = np.empty((B, S, D), np.float32)
    for b in range(B):
        full[b] = (outs[2 * b] + outs[2 * b + 1]).T + const
    return full, res


def kernel(**inputs):
    full, _ = run(inputs)
    return full

